# revision 27
# baseline (speedup 1.0000x reference)
"""Attention (B=4, S=4096, W=512, E=64) on 8 TRN2 NeuronCores.

Sharding: core c handles batch b = c//2, query half h = c%2 (2048 queries).
Each core receives x[b]^T as bf16 with the key/value columns ordered so that
this core's query half occupies columns [0, 2048) (softmax over keys is
permutation invariant as long as K and V share the order). K/V are computed
for the full sequence locally; flash-style attention over the core's query
half. No collectives.

Math simplifications vs the reference:
  - K bias bk drops out of softmax entirely (its score contribution beyond
    bq.K is a per-query constant).
  - V bias bv commutes with the softmax average, added on the host.
  - Only the Q bias bq remains on-device (fused into the Q projection
    evacuation on ScalarE as an Identity-activation with per-partition bias).

Structure (two phases, PSUM banks time-shared between them):
  Phase 1 — projections (PSUM pool psP, 2 banks):
    ~11 dependency-free warmup matmuls on uninitialized SBUF release the PE
    HAM clock-gate during the framework preamble + x DMA-in; x^T streamed
    via 16 HWDGE transfers (mostly sync-issued so ScalarE is free);
    kv = V^T (p0:64) / K^T (p64:128) and Q^T duplicated on both halves,
    PSUM evacuations on ScalarE; K^T replicated to partitions 0:64 by DMA;
    V' transposes run one chunk behind their projection so the evacuation
    latency is hidden.
  Phase 2 — attention (psS 3x2 + psZ 1 + psA 1 = 8 banks), four passes of
  one 512-query chunk; per key-tile pair kp:
    scores S^T, two k-tiles packed in PE row groups (e=64);
    exp alternates engines per kp: ScalarE exact (ACT table) on even kp,
    VectorE on odd kp via a Schraudolph bit-trick (one tensor_scalar
    mul-add fp32->int8 whose bits are e4m3 ~= exp, ~4% max error which
    softmax normalization cancels to ~1e-3 in Z);
    AV is a single fp8e4 DoubleRow matmul (contraction 256 over the pair,
    V' carrying a ones column so row 64 accumulates the denominator);
    triple-buffered score PSUM keeps the PE two pairs ahead of the exp.
  Normalization of chunk i overlaps pass i+1 (PE-transpose, batched
  reciprocal, multiplies split across ScalarE/VectorE, DMA out).
"""

import numpy as np
import ml_dtypes

import concourse.bass as bass
import concourse.mybir as mybir
import concourse.tile as tile
from concourse import bacc
from concourse.bass import ts
from concourse.masks import make_identity
from concourse.bass_utils import run_bass_kernel_spmd

BF16 = mybir.dt.bfloat16
F32 = mybir.dt.float32
F8 = mybir.dt.float8e4
I8 = mybir.dt.int8
NP_BF16 = ml_dtypes.bfloat16

B = 4
S_FULL = 4096
W = 512
E = 64
TQ = 2048  # queries per core
WT = W // 128  # 4 contraction tiles
KT = S_FULL // 128  # 32 key tiles
KP = KT // 2  # 16 key-tile pairs
NCH = S_FULL // 512  # 8 projection chunks
QC = TQ // 512  # 4 query chunks of 512
SCALE = 0.125  # 1/sqrt(E)
N_WARMUP = 8

# Schraudolph exp bit-trick (fp8 e4m3): bits8(exp(SCALE*s)) ~= round(A*s+B);
# B tuned on the key-tile-alternating split (absmax rel err 1.52e-2).
A_SCH8 = float(SCALE * np.log2(np.e) * 8.0)
B_SCH8 = float(7.0 * 8.0 - 0.6)
MPAD = 80  # V' columns padded so the DoubleRow Ko step is 16B-aligned

_NC_CACHE = {}


def build_nc():
    nc = bacc.Bacc("TRN2", target_bir_lowering=False)
    xT = nc.dram_tensor("xT", [W, S_FULL], BF16, kind="ExternalInput")
    wqq = nc.dram_tensor("wqq", [128, W], BF16, kind="ExternalInput")
    wkv = nc.dram_tensor("wkv", [128, W], BF16, kind="ExternalInput")
    bqq = nc.dram_tensor("bqq", [128, 1], F32, kind="ExternalInput")
    y = nc.dram_tensor("y", [TQ, E], F32, kind="ExternalOutput")

    MULT = mybir.AluOpType.mult
    ADD = mybir.AluOpType.add
    EXP = mybir.ActivationFunctionType.Exp
    DR = mybir.MatmulPerfMode.DoubleRow

    with tile.TileContext(nc) as tc:
        with (
            tc.tile_pool(name="const", bufs=1) as const,
            tc.tile_pool(name="pp0", bufs=3) as pp0,
            tc.tile_pool(name="pp1", bufs=3) as pp1,
            tc.tile_pool(name="zsb", bufs=2) as zsbp,
            tc.tile_pool(name="small", bufs=2) as small,
            tc.tile_pool(name="outp", bufs=2) as outp,
        ):
            # weights on ScalarE, bias on GpSimd so the sync queue is pure x
            wkv_sb = const.tile([128, WT, 128], BF16)
            wqq_sb = const.tile([128, WT, 128], BF16)
            nc.scalar.dma_start(
                out=wkv_sb, in_=wkv[:, :].rearrange("p (t m) -> p t m", t=WT)
            )
            nc.scalar.dma_start(
                out=wqq_sb, in_=wqq[:, :].rearrange("p (t m) -> p t m", t=WT)
            )
            bqq_sb = const.tile([128, 1], F32)
            nc.gpsimd.dma_start(out=bqq_sb, in_=bqq[:, :])

            # x^T: first-chunk pieces split sync/scalar for earliest arrival,
            # the rest all on sync (ScalarE must be free for evacuations)
            xt_sb = const.tile([128, WT, S_FULL], BF16)
            for ch2 in range(NCH // 2):
                for t in range(WT):
                    nc.sync.dma_start(
                        out=xt_sb[:, t, ts(ch2, 1024)],
                        in_=xT[t * 128:(t + 1) * 128, ts(ch2, 1024)],
                    )

            ident_bf = const.tile([64, 64], BF16)
            make_identity(nc, ident_bf)
            ident_f32 = const.tile([E + 1, E + 1], F32)
            make_identity(nc, ident_f32)

            kv_sb = const.tile([128, S_FULL], BF16)  # V^T (p0:64) / K^T (p64:)
            krep = const.tile([64, S_FULL], BF16)  # K^T replica on p0:64
            qtpair = const.tile([128, TQ], BF16)  # Q^T on both halves
            # V' = [V | 1 | pad] in fp8, DoubleRow-interleaved per k-pair
            vp_sb = const.tile([128, KP, 2, MPAD], F8)
            nc.gpsimd.memset(vp_sb, 1.0)
            # junk operand for warmup matmuls; memset on the idle VectorE is
            # the only dependency so the PE can start during the x stream
            wu_src = const.tile([128, 512], BF16)
            nc.vector.memset(wu_src, 1.0)

            # ---- phase 1: projections (2 PSUM banks, then released) ----
            with tc.tile_pool(name="psP", bufs=4, space="PSUM") as psP:
                wu = psP.tile([128, 512], F32, tag="pj", name="wu")
                for _ in range(N_WARMUP):
                    nc.tensor.matmul(
                        wu, wu_src[:, 0:128], wu_src, start=True, stop=True
                    )

                def emit_kv_mm(ch):
                    ps = psP.tile([128, 512], F32, tag="pj", name=f"pskv{ch}")
                    for t in range(WT):
                        nc.tensor.matmul(
                            ps,
                            wkv_sb[:, t, :],
                            xt_sb[:, t, ts(ch, 512)],
                            start=(t == 0),
                            stop=(t == WT - 1),
                        )
                    nc.scalar.copy(kv_sb[:, ts(ch, 512)], ps)
                    nc.gpsimd.dma_start(
                        out=krep[:, ts(ch, 512)],
                        in_=kv_sb[64:128, ts(ch, 512)],
                    )

                def emit_vt(ch):
                    # V^T -> V' (4 PE transposes, one batched fp8 cast),
                    # emitted a chunk late so the evacuation wait is hidden;
                    # the last two casts go on the then-idle ScalarE so the
                    # first score matmul's bank-reuse WAR clears earlier
                    vt = psP.tile([128, 4, E], BF16, tag="pj", name=f"vt{ch}")
                    for j in range(4):
                        nc.tensor.transpose(
                            vt[:, j, :],
                            kv_sb[0:64, ts(4 * ch + j, 128)],
                            ident_bf,
                        )
                    dst = vp_sb[:, 2 * ch:2 * ch + 2, :, 0:E]
                    if ch >= NCH - 2:
                        nc.scalar.copy(dst, vt)
                    else:
                        nc.vector.tensor_copy(dst, vt)

                def emit_q(ch):
                    ps = psP.tile([128, 512], F32, tag="pj", name=f"psq{ch}")
                    for t in range(WT):
                        nc.tensor.matmul(
                            ps,
                            wqq_sb[:, t, :],
                            xt_sb[:, t, ts(ch, 512)],
                            start=(t == 0),
                            stop=(t == WT - 1),
                        )
                    nc.scalar.add(qtpair[:, ts(ch, 512)], ps, bqq_sb)

                for ch in range(NCH):
                    emit_kv_mm(ch)
                    if ch < QC:
                        emit_q(ch)
                    if ch >= 1:
                        emit_vt(ch - 1)
                emit_vt(NCH - 1)

            # ---- phase 2: attention (6 + 1 + 1 PSUM banks) ----
            with (
                tc.tile_pool(name="psS", bufs=3, space="PSUM") as psS,
                tc.tile_pool(name="psZ", bufs=1, space="PSUM") as psZ,
                tc.tile_pool(name="psA", bufs=1, space="PSUM") as psA,
            ):
                norm_state = {}

                def norm_copy(zp, qc):
                    zsb = zsbp.tile(
                        [E + 1, 512], F32, tag="zsb", name=f"zsb{qc}"
                    )
                    nc.scalar.copy(zsb, zp[0:E + 1, :])
                    norm_state[qc] = zsb

                def norm_rest(qc, out_sync=False):
                    zsb = norm_state[qc]
                    zt = psA.tile(
                        [128, 4, E + 1], F32, tag="zt", name=f"zt{qc}"
                    )
                    for sub in range(4):
                        nc.tensor.transpose(
                            zt[:, sub, :], zsb[:, ts(sub, 128)], ident_f32
                        )
                    o_sb = outp.tile([128, 4, E], F32, tag="o", name=f"o{qc}")
                    r4 = small.tile([128, 4, 1], F32, tag="r", name=f"r{qc}")
                    nc.vector.reciprocal(r4, zt[:, :, E:E + 1])
                    for sub in range(4):
                        if sub < 2:
                            nc.scalar.mul(
                                o_sb[:, sub, :], zt[:, sub, 0:E],
                                r4[:, sub, 0:1],
                            )
                        else:
                            nc.vector.tensor_scalar_mul(
                                o_sb[:, sub, :], zt[:, sub, 0:E],
                                r4[:, sub, 0:1],
                            )
                    y_ap = y[ts(qc, 512), :].rearrange("(t p) e -> p t e", t=4)
                    eng = nc.sync if out_sync else nc.gpsimd
                    eng.dma_start(out=y_ap, in_=o_sb)

                def emit_av(zp, kp, rhs):
                    nc.tensor.matmul(
                        zp, vp_sb[:, kp, :, :], rhs,
                        start=(kp == 0), stop=(kp == KP - 1), perf_mode=DR,
                    )

                prev = None
                for qc in range(QC):
                    zp = psZ.tile([MPAD, 512], F32, tag="z", name=f"zp{qc}")
                    pend = []
                    for kp in range(KP):
                        if prev is not None:
                            if kp == 0:
                                norm_copy(*prev)
                            elif kp == 1:
                                norm_rest(prev[1])
                        ka, kb = 2 * kp, 2 * kp + 1
                        sp = psS.tile(
                            [128, 1024], F32, tag="sp", name=f"sp{qc}_{kp}"
                        )
                        nc.tensor.matmul(
                            sp[:, 0:512], krep[:, ts(ka, 128)],
                            qtpair[0:64, ts(qc, 512)], start=True, stop=True,
                        )
                        nc.tensor.matmul(
                            sp[:, 512:1024], kv_sb[64:128, ts(kb, 128)],
                            qtpair[64:128, ts(qc, 512)],
                            start=True, stop=True,
                        )
                        if kp % 2 == 0:
                            p = pp0.tile(
                                [128, 2, 512], F8, tag="p0",
                                name=f"p{qc}_{kp}",
                            )
                            nc.scalar.activation(p, sp, EXP, scale=SCALE)
                            rhs = p
                        else:
                            p = pp1.tile(
                                [128, 2, 512], I8, tag="p1",
                                name=f"p{qc}_{kp}",
                            )
                            nc.vector.tensor_scalar(
                                p, sp, A_SCH8, B_SCH8, MULT, ADD
                            )
                            rhs = p.bitcast(F8)
                        if len(pend) == 2:
                            emit_av(zp, *pend.pop(0))
                        pend.append((kp, rhs))
                    for args in pend:
                        emit_av(zp, *args)
                    prev = (zp, qc)
                norm_copy(*prev)
                norm_rest(prev[1], out_sync=True)
    nc.compile()
    return nc


def get_nc():
    if "nc" not in _NC_CACHE:
        _NC_CACHE["nc"] = build_nc()
    return _NC_CACHE["nc"]


def make_in_maps(x, Wq, bq, Wk, bk, Wv, bv):
    x = np.asarray(x, dtype=np.float32)
    Wq = np.asarray(Wq, dtype=np.float32)
    Wk = np.asarray(Wk, dtype=np.float32)
    Wv = np.asarray(Wv, dtype=np.float32)
    bq = np.asarray(bq, dtype=np.float32)

    def _pmajor(w):  # [512, 128] w-major -> [128, 512] partition-major
        return np.ascontiguousarray(
            w.reshape(4, 128, 128).transpose(1, 0, 2).reshape(128, 512)
        )

    wkv_host = _pmajor(
        np.concatenate([Wv.T, Wk.T], axis=1).astype(NP_BF16)
    )
    wqq_host = _pmajor(
        np.concatenate([Wq.T, Wq.T], axis=1).astype(NP_BF16)
    )
    bqq_host = np.ascontiguousarray(
        np.concatenate([bq, bq]).reshape(128, 1)
    ).astype(np.float32)

    in_maps = []
    for c in range(8):
        b, h = c // 2, c % 2
        xT_b = np.asarray(x[b].T, dtype=NP_BF16)
        if h == 1:  # put this core's query half into columns [0, 2048)
            xT_b = np.concatenate([xT_b[:, TQ:], xT_b[:, :TQ]], axis=1)
        in_maps.append(
            {
                "xT": np.ascontiguousarray(xT_b),
                "wqq": wqq_host,
                "wkv": wkv_host,
                "bqq": bqq_host,
            }
        )
    return in_maps


def assemble(results, bv):
    bv = np.asarray(bv, dtype=np.float32)
    out = np.empty((B, S_FULL, E), dtype=np.float32)
    for c in range(8):
        b, h = c // 2, c % 2
        out[b, h * TQ:(h + 1) * TQ, :] = results[c]["y"] + bv
    return out


def kernel(x, Wq, bq, Wk, bk, Wv, bv, **_unused):
    in_maps = make_in_maps(x, Wq, bq, Wk, bk, Wv, bv)
    nc = get_nc()
    res = run_bass_kernel_spmd(nc, in_maps, core_ids=list(range(8)))
    return assemble(res.results, bv)
